# revision 1
# baseline (speedup 1.0000x reference)
"""Trainium2 Bass kernel for nn_CaptionModel (GRU caption decoder).

Model: h0 = feat; x0 = embed[<SOS>]; 200 GRU steps where the *output hidden
state is fed back as the next input* (x_t = h_t for t >= 1), then a linear
projection of every hidden state to vocab logits, output [B, V, T].

Because x_t == h_t for t >= 1, the two GRU matmuls fuse into one:
  G = h @ Wc.T + bc  with  Wc = [w_ih_r+w_hh_r; w_ih_z+w_hh_z; w_ih_n; w_hh_n]
  r = sig(G0), z = sig(G1), n = tanh(G2 + r*G3), h' = (1-z)*n + z*h
Step 0 folds x0 through w_ih into a modified bias (gi0) and uses w_hh only.

Sharding: pure data parallelism, batch 256 -> 32 per core on 8 cores,
weights replicated. Per-core layout: batch on PSUM partitions (M=32),
gates on the free dim, contraction H=512 as 4 k-chunks of 128 with the
transposed hidden state as the (tiny) stationary operand and the f32r
weights streamed as the moving operand (1 cyc/col at N>=512). Biases are
injected via K=1 matmuls of a ones-row. The new h is transposed back with
4 PE-transposes per step straight into a history buffer that serves as
(a) the next step's stationary operand and (b) the projection's rhs.
"""

import os
from contextlib import ExitStack

import numpy as np

import concourse.bass as bass
import concourse.tile as tile
from concourse import bacc, mybir
from concourse.bass_utils import run_bass_kernel_spmd

B, H, VOCAB = 256, 512, 100
STEPS = int(os.environ.get("KERNEL_STEPS", "200"))
NCORES = 8
BD = B // NCORES  # 32
KC = H // 128  # 4 k-chunks
G4 = 4 * H  # 2048 fused gate width
PB = 2  # batch rows per projection chunk
F32 = mybir.dt.float32
F32R = mybir.dt.float32r


def _build(steps: int):
    nc = bacc.Bacc("TRN2", target_bir_lowering=False, debug=False,
                   num_devices=NCORES)

    feat_d = nc.dram_tensor("feat", [BD, H], F32, kind="ExternalInput").ap()
    wct_d = nc.dram_tensor("wct", [KC, 128, G4], F32R, kind="ExternalInput").ap()
    wc0t_d = nc.dram_tensor("wc0t", [KC, 128, 3 * H], F32R, kind="ExternalInput").ap()
    bct_d = nc.dram_tensor("bct", [1, G4], F32R, kind="ExternalInput").ap()
    bc0t_d = nc.dram_tensor("bc0t", [1, G4], F32R, kind="ExternalInput").ap()
    projt_d = nc.dram_tensor("projt", [KC, 128, VOCAB], F32R, kind="ExternalInput").ap()
    projb_d = nc.dram_tensor("projb", [VOCAB, 1], F32, kind="ExternalInput").ap()
    ident_d = nc.dram_tensor("ident", [32, 32], F32, kind="ExternalInput").ap()
    ones_d = nc.dram_tensor("ones", [1, BD], F32R, kind="ExternalInput").ap()
    out_d = nc.dram_tensor("out", [BD, VOCAB, steps], F32, kind="ExternalOutput").ap()

    SIG = mybir.ActivationFunctionType.Sigmoid
    TANH = mybir.ActivationFunctionType.Tanh

    with tile.TileContext(nc) as tc, ExitStack() as ctx:
        singles = ctx.enter_context(tc.tile_pool(name="singles", bufs=1))
        hpool = ctx.enter_context(tc.tile_pool(name="h", bufs=2))
        work = ctx.enter_context(tc.tile_pool(name="work", bufs=1))

        # --- resident constants / weights ---
        ident_s = singles.tile([32, 32], F32)
        nc.sync.dma_start(out=ident_s, in_=ident_d)
        ones_s = singles.tile([1, BD], F32R)
        nc.sync.dma_start(out=ones_s, in_=ones_d)
        bc0t_s = singles.tile([1, G4], F32R)
        nc.sync.dma_start(out=bc0t_s, in_=bc0t_d)
        bct_s = singles.tile([1, G4], F32R)
        nc.sync.dma_start(out=bct_s, in_=bct_d)
        wc0t_s = singles.tile([128, KC, 3 * H], F32R)
        nc.sync.dma_start(out=wc0t_s, in_=wc0t_d.rearrange("c p n -> p c n"))
        wct_s = singles.tile([128, KC, G4], F32R)
        nc.sync.dma_start(out=wct_s, in_=wct_d.rearrange("c p n -> p c n"))
        projt_s = singles.tile([128, KC, VOCAB], F32R)
        nc.sync.dma_start(out=projt_s, in_=projt_d.rearrange("c p n -> p c n"))
        projb_s = singles.tile([VOCAB, 1], F32)
        nc.sync.dma_start(out=projb_s, in_=projb_d)

        # history of transposed hidden states: hist[c][p, b, t] = h_t[b, c*128+p]
        hist = [singles.tile([128, BD, steps], F32R, tag=f"hist{c}", name=f"hist{c}")
                for c in range(KC)]
        hT0_s = singles.tile([128, KC, BD], F32R)

        with tc.tile_pool(name="gpsum", bufs=1, space="PSUM") as gpool, \
             tc.tile_pool(name="tpsum", bufs=2, space="PSUM") as tpool:

            # --- h0 = feat; build transposed h0 ---
            h_first = hpool.tile([BD, H], F32, tag="h")
            nc.sync.dma_start(out=h_first, in_=feat_d)
            for c in range(KC):
                tp = tpool.tile([128, BD], F32, tag="tp")
                nc.tensor.transpose(tp, h_first[:, c * 128:(c + 1) * 128], ident_s)
                nc.scalar.copy(out=hT0_s[:, c, :], in_=tp)

            h_prev = h_first

            def emit_step(t, h_mid):
                """Emit step t's matmuls (interleaved with the transposes of
                h_mid = h_{t-1} into hist[.., t-1]) and the gate math,
                returning h_t. For t==0, h_mid is None (hT0 pre-built)."""
                bias_s = bc0t_s if t == 0 else bct_s

                def lhsT(c):
                    return hT0_s[:, c, :] if t == 0 else hist[c][:, :, t - 1]

                def wslice(g, lo, hi):
                    if t == 0:
                        col0 = {0: 0, 1: 512, 3: 1024}[g]
                        return wc0t_s, col0 + lo, col0 + hi
                    return wct_s, g * 512 + lo, g * 512 + hi

                # six psum accumulators, one bank each
                r_ps = gpool.tile([BD, 512], F32, tag="r_ps")
                z_ps = gpool.tile([BD, 512], F32, tag="z_ps")
                hnA_ps = gpool.tile([BD, 256], F32, tag="hnA_ps")
                hnB_ps = gpool.tile([BD, 256], F32, tag="hnB_ps")
                inA_ps = gpool.tile([BD, 256], F32, tag="inA_ps")
                inB_ps = gpool.tile([BD, 256], F32, tag="inB_ps")

                def tc(c):
                    if h_mid is None:
                        return
                    tp = tpool.tile([128, BD], F32, tag="tp")
                    nc.tensor.transpose(
                        tp, h_mid[:, c * 128:(c + 1) * 128], ident_s)
                    nc.scalar.copy(out=hist[c][:, :, t - 1], in_=tp)

                def kmm(ps, g, lo, hi, c):
                    w_ap, wlo, whi = wslice(g, lo, hi)
                    nc.tensor.matmul(ps, lhsT(c), w_ap[:, c, wlo:whi],
                                     start=False, stop=(c == KC - 1))

                def bias_mm(ps, g, lo, hi, stop=False):
                    nc.tensor.matmul(ps, ones_s,
                                     bias_s[:, g * 512 + lo:g * 512 + hi],
                                     start=True, stop=stop)

                in_bias_only = (t == 0)
                bias_mm(r_ps, 0, 0, 512)
                bias_mm(z_ps, 1, 0, 512)
                # interleave: transpose chunk c of h_{t-1}, then the k=c
                # matmuls of r and z that consume it
                for c in range(KC):
                    tc(c)
                    kmm(r_ps, 0, 0, 512, c)
                    kmm(z_ps, 1, 0, 512, c)
                for hn_ps, in_ps, lo, hi in ((hnA_ps, inA_ps, 0, 256),
                                             (hnB_ps, inB_ps, 256, 512)):
                    bias_mm(in_ps, 2, lo, hi, stop=in_bias_only)
                    if not in_bias_only:
                        for c in range(KC):
                            kmm(in_ps, 2, lo, hi, c)
                    bias_mm(hn_ps, 3, lo, hi)
                    for c in range(KC):
                        kmm(hn_ps, 3, lo, hi, c)

                r_s = work.tile([BD, H], F32, tag="r")
                nc.scalar.activation(r_s[:, 0:256], r_ps[:, 0:256], SIG)
                nc.scalar.activation(r_s[:, 256:512], r_ps[:, 256:512], SIG)
                z_s = work.tile([BD, H], F32, tag="z")
                nc.scalar.activation(z_s, z_ps, SIG)
                z1m_s = work.tile([BD, H], F32, tag="z1m")
                nc.scalar.activation(z1m_s, z_ps, SIG, scale=-1.0)
                u_s = work.tile([BD, H], F32, tag="u")
                nc.gpsimd.tensor_mul(u_s, z_s, h_prev)

                a_s = work.tile([BD, H], F32, tag="a")
                b_s = work.tile([BD, H], F32, tag="b")
                n_s = work.tile([BD, H], F32, tag="n")
                h_new = hpool.tile([BD, H], F32, tag="h")

                e_s = work.tile([BD, H], F32, tag="e")

                def npath_chain(hn_ps, in_ps, lo, skip_a=False):
                    qs = slice(lo, lo + 256)
                    if not skip_a:
                        nc.vector.tensor_mul(a_s[:, qs], r_s[:, qs], hn_ps)
                    nc.vector.tensor_add(b_s[:, qs], a_s[:, qs], in_ps)
                    nc.scalar.activation(n_s[:, qs], b_s[:, qs], TANH)
                    nc.vector.tensor_mul(e_s[:, qs], z1m_s[:, qs], n_s[:, qs])
                    nc.vector.tensor_add(h_new[:, qs], u_s[:, qs], e_s[:, qs])

                # fast-path the first 128-col quarter: it alone gates
                # T0 -> c0 -> next step's k0 matmuls
                for q in (0, 1):
                    qs = slice(q * 128, (q + 1) * 128)
                    nc.vector.tensor_mul(a_s[:, qs], r_s[:, qs], hnA_ps[:, qs])
                    nc.vector.tensor_add(b_s[:, qs], a_s[:, qs], inA_ps[:, qs])
                    nc.scalar.activation(n_s[:, qs], b_s[:, qs], TANH)
                    nc.vector.tensor_mul(e_s[:, qs], z1m_s[:, qs], n_s[:, qs])
                    nc.vector.tensor_add(h_new[:, qs], u_s[:, qs], e_s[:, qs])
                for q in (2, 3):
                    qs = slice(q * 128, (q + 1) * 128)
                    ps_q = slice((q - 2) * 128, (q - 1) * 128)
                    nc.vector.tensor_mul(a_s[:, qs], r_s[:, qs], hnB_ps[:, ps_q])
                    nc.vector.tensor_add(b_s[:, qs], a_s[:, qs], inB_ps[:, ps_q])
                    nc.scalar.activation(n_s[:, qs], b_s[:, qs], TANH)
                    nc.vector.tensor_mul(e_s[:, qs], z1m_s[:, qs], n_s[:, qs])
                    nc.vector.tensor_add(h_new[:, qs], u_s[:, qs], e_s[:, qs])
                return h_new

            reps = int(os.environ.get("KERNEL_REPS", "1"))
            for rep in range(reps):
                for t in range(steps):
                    if rep == 0 and t == 0:
                        h_new = emit_step(0, None)
                    elif t == 0:
                        continue  # bench-only replication skips step 0
                    else:
                        h_new = emit_step(t, h_prev)
                    h_prev = h_new

            # final state still needs transposing into hist[.., steps-1]
            for c in range(KC):
                tp = tpool.tile([128, BD], F32, tag="tp")
                nc.tensor.transpose(tp, h_prev[:, c * 128:(c + 1) * 128], ident_s)
                nc.scalar.copy(out=hist[c][:, :, steps - 1], in_=tp)

        # --- projection: logits[v, b, t] = proj_w @ h + proj_b ---
        with tc.tile_pool(name="ppsum", bufs=2, space="PSUM") as ppool, \
             tc.tile_pool(name="stage", bufs=2) as spool:
            NW = PB * steps
            for j in range(BD // PB):
                P = ppool.tile([VOCAB, NW], F32, tag="P")
                for c in range(KC):
                    rhs = hist[c][:, j * PB:(j + 1) * PB, :].rearrange(
                        "p b t -> p (b t)")
                    nc.tensor.matmul(P, projt_s[:, c, :], rhs,
                                     start=(c == 0), stop=(c == KC - 1))
                stage = spool.tile([VOCAB, NW], F32, tag="stage")
                nc.vector.tensor_scalar_add(stage, P, projb_s)
                nc.sync.dma_start(
                    out=out_d[j * PB:(j + 1) * PB].rearrange("b v t -> v b t"),
                    in_=stage.rearrange("p (b t) -> p b t", b=PB))

    nc.compile()
    return nc


_CACHE = {}


def _get_nc(steps: int):
    if steps not in _CACHE:
        _CACHE[steps] = _build(steps)
    return _CACHE[steps]


def _prep_inputs(feat, embed_table, w_ih, w_hh, b_ih, b_hh, proj_w, proj_b):
    f32 = np.float32
    w_ih = np.asarray(w_ih, f32)
    w_hh = np.asarray(w_hh, f32)
    b_ih = np.asarray(b_ih, f32)
    b_hh = np.asarray(b_hh, f32)
    Wc = np.concatenate([w_ih[:H] + w_hh[:H], w_ih[H:2 * H] + w_hh[H:2 * H],
                         w_ih[2 * H:], w_hh[2 * H:]], 0)  # [4H, H]
    bc = np.concatenate([b_ih[:H] + b_hh[:H], b_ih[H:2 * H] + b_hh[H:2 * H],
                         b_ih[2 * H:], b_hh[2 * H:]], 0)  # [4H]
    x0 = np.asarray(embed_table, f32)[0]
    gi0 = w_ih @ x0 + b_ih
    bc0 = np.concatenate([gi0[:H] + b_hh[:H], gi0[H:2 * H] + b_hh[H:2 * H],
                          gi0[2 * H:], b_hh[2 * H:]], 0)
    Wc0 = np.concatenate([w_hh[:H], w_hh[H:2 * H], w_hh[2 * H:]], 0)  # [3H, H]

    common = {
        "wct": np.ascontiguousarray(Wc.T.reshape(KC, 128, G4)),
        "wc0t": np.ascontiguousarray(Wc0.T.reshape(KC, 128, 3 * H)),
        "bct": bc.reshape(1, G4),
        "bc0t": bc0.reshape(1, G4),
        "projt": np.ascontiguousarray(
            np.asarray(proj_w, f32).T.reshape(KC, 128, VOCAB)),
        "projb": np.asarray(proj_b, f32).reshape(VOCAB, 1),
        "ident": np.eye(32, dtype=f32),
        "ones": np.ones((1, BD), f32),
    }
    feat = np.asarray(feat, f32)
    return [dict(common, feat=np.ascontiguousarray(feat[i * BD:(i + 1) * BD]))
            for i in range(NCORES)]


def kernel(feat, embed_table, w_ih, w_hh, b_ih, b_hh, proj_w, proj_b,
           _trace=False):
    nc = _get_nc(STEPS)
    in_maps = _prep_inputs(feat, embed_table, w_ih, w_hh, b_ih, b_hh,
                           proj_w, proj_b)
    res = run_bass_kernel_spmd(nc, in_maps, list(range(NCORES)), trace=_trace)
    out = np.concatenate([res.results[i]["out"] for i in range(NCORES)], 0)
    if _trace:
        kernel.last_exec_time_ns = res.exec_time_ns
        kernel.last_results = res
    return out



# revision 14
# speedup vs baseline: 1.8583x; 1.8583x over previous
"""Trainium2 Bass kernel for nn_CaptionModel (GRU caption decoder).

Model: h0 = feat; x0 = embed[<SOS>]; 200 GRU steps where the output hidden
state is fed back as the next input (x_t = h_t for t >= 1), then a linear
projection of every hidden state to vocab logits, output [B, V, T].

Since x_t == h_t for t >= 1 the two GRU matmuls fuse into one 2048-wide
gate matmul G = h @ Wc.T + bc with Wc = [w_ih_r+w_hh_r; w_ih_z+w_hh_z;
w_hh_n; w_ih_n], gates r = sig(G0), z = sig(G1), n = tanh(G3 + r*G2),
h' = (1-z)*n + z*h.

Layout: GATE-MAJOR, fp16 matmuls. Each core holds batch BD=32. The PE
computes G^T [2048 gates -> 16 chunks of 128 partitions, 32 batch free]
with the 128x128 weight blocks as stationary operands and the (tiny)
hidden state as the moving operand: 16 bias rows + 128 weight matmuls of
32 moving rows each (~0.6us PE busy vs ~3.5us for weight-moving layouts;
stationary loads are pipelined). h' = u + e is never materialized on the
critical path: u = (1-z)*h (ready early) and e = (1-z)*n are fed to the
PE as TWO accumulating moving operands, removing the final join from the
recurrence cycle. z1m = 1-z comes directly from sigmoid(-zpre)
(scale=-1), and the u path (t1 = z1m*h, u = h - t1) runs off-chain on
Pool/DVE. The vocab projection is interleaved every 4 steps (lagged) to
fill PE idle time; hidden history is kept gate-major fp16 and projected
with the same stationary-weight trick.

Sharding: pure data parallelism, batch 256 -> 32 per core on 8 cores,
weights replicated.
"""

import os
from contextlib import ExitStack

import numpy as np

import concourse.bass as bass
import concourse.tile as tile
from concourse import bacc, mybir
from concourse.bass_utils import run_bass_kernel_spmd

B, H, VOCAB = 256, 512, 100
STEPS = int(os.environ.get("KERNEL_STEPS", "200"))
NCORES = 8
BD = B // NCORES  # 32
F16 = mybir.dt.float16
F32 = mybir.dt.float32
SIG = mybir.ActivationFunctionType.Sigmoid
TANH = mybir.ActivationFunctionType.Tanh

# gate order in both the weight blocks and the PSUM column regions
# r [0:128] z [128:256] hn [256:384] in [384:512]
GATES = ("r", "z", "hn", "in")
GI = {g: i for i, g in enumerate(GATES)}
PROJ_EVERY = 4


def _blk(g, q, c):
    return ((GI[g] * 4 + q) * 4 + c) * 128


def _build(steps: int):
    nc = bacc.Bacc("TRN2", target_bir_lowering=False, debug=False,
                   num_devices=NCORES)
    T1 = steps + 1

    wst_d = nc.dram_tensor("wst", [128, 64 * 128], F16, kind="ExternalInput").ap()
    wst0_d = nc.dram_tensor("wst0", [128, 32 * 128], F16, kind="ExternalInput").ap()
    h0_d = nc.dram_tensor("h0", [128, 128], F16, kind="ExternalInput").ap()
    bt_d = nc.dram_tensor("bt", [1, 2048], F16, kind="ExternalInput").ap()
    bt0_d = nc.dram_tensor("bt0", [1, 2048], F16, kind="ExternalInput").ap()
    ones_d = nc.dram_tensor("ones", [1, BD], F16, kind="ExternalInput").ap()
    pjt_d = nc.dram_tensor("pjt", [128, 4 * VOCAB], F16, kind="ExternalInput").ap()
    pjb_d = nc.dram_tensor("pjb", [VOCAB, 1], F32, kind="ExternalInput").ap()
    out_d = nc.dram_tensor("out", [BD, VOCAB, steps], F32,
                           kind="ExternalOutput").ap()

    with tile.TileContext(nc) as tc, ExitStack() as ctx:
        sg = ctx.enter_context(tc.tile_pool(name="sg", bufs=1))
        wk = ctx.enter_context(tc.tile_pool(name="wk", bufs=2))

        wst = sg.tile([128, 64 * 128], F16)
        nc.sync.dma_start(out=wst, in_=wst_d)
        wst0 = sg.tile([128, 32 * 128], F16)
        nc.sync.dma_start(out=wst0, in_=wst0_d)
        bt = sg.tile([1, 2048], F16)
        nc.sync.dma_start(out=bt, in_=bt_d)
        bt0 = sg.tile([1, 2048], F16)
        nc.sync.dma_start(out=bt0, in_=bt0_d)
        ones = sg.tile([1, BD], F16)
        nc.sync.dma_start(out=ones, in_=ones_d)
        pjt = sg.tile([128, 4 * VOCAB], F16)
        nc.sync.dma_start(out=pjt, in_=pjt_d)
        pjb = sg.tile([VOCAB, 1], F32)
        nc.sync.dma_start(out=pjb, in_=pjb_d)
        hist = sg.tile([128, 4, T1, BD], F16, name="hist")
        nc.sync.dma_start(out=hist[:, :, 0, :],
                          in_=h0_d.rearrange("p (q b) -> p q b", q=4))
        stage = sg.tile([VOCAB, BD * steps], F32, name="stage")

        with tc.tile_pool(name="gps", bufs=1, space="PSUM") as gpool, \
             tc.tile_pool(name="pps", bufs=2, space="PSUM") as ppool:
            G = [gpool.tile([128, 512], F32, tag=f"G{i}", name=f"G{i}")
                 for i in range(2)]

            # One PSUM accumulation group per G bank per step: start=True
            # zeroes the WHOLE 2KB zero region, so only the very first
            # matmul into the bank may carry start, and only the very last
            # carries stop.
            def bias_mms(t, bias):
                g = G[t % 2]
                first = True
                for gate in GATES:
                    for q in range(4):
                        nc.tensor.matmul(
                            g[:, GI[gate] * 128 + q * BD:
                              GI[gate] * 128 + (q + 1) * BD],
                            bias[:, GI[gate] * 512 + q * 128:
                                 GI[gate] * 512 + (q + 1) * 128],
                            ones, start=first, stop=False,
                            skip_group_check=True)
                        first = False

            def w_mms(t, w, rhs4, last, skip=(), cs=(0, 1, 2, 3)):
                g = G[t % 2]
                emitted = []
                for gate in GATES:
                    if gate in skip:
                        continue
                    for q in range(4):
                        for c in cs:
                            emitted.append((gate, q, c))
                for i, (gate, q, c) in enumerate(emitted):
                    col = GI[gate] * 128 + q * BD
                    if w is wst0 and gate in ("r", "z"):
                        wt, base = wst0, ((GI[gate] * 4 + q) * 4 + c) * 128
                    else:
                        wt, base = wst, _blk(gate, q, c)
                    nc.tensor.matmul(
                        g[:, col:col + BD],
                        wt[:, base:base + 128],
                        rhs4[c], start=False,
                        stop=(last and i == len(emitted) - 1),
                        skip_group_check=True)

            def proj_rows(r0, nrows):
                # P free dim iterates (b, t) so the stage (b-major,
                # t-minor) write and the final DMA stay contiguous in t
                Pfull = ppool.tile([VOCAB, 512], F32, tag="P", name="Pfull")
                P = Pfull[:, 0:BD * nrows]
                for c in range(4):
                    rhs = hist[:, c, r0:r0 + nrows, :].rearrange(
                        "p t b -> p b t")
                    nc.tensor.matmul(P, pjt[:, c * VOCAB:(c + 1) * VOCAB], rhs,
                                     start=(c == 0), stop=(c == 3))
                st_sl = stage.rearrange("p (b t) -> p b t", b=BD)[
                    :, :, r0 - 1:r0 - 1 + nrows]
                nc.vector.tensor_scalar_add(
                    st_sl, P.rearrange("p (b t) -> p b t", b=BD), pjb)

            # t=0: bias0 + w_hh matmuls on h0 (in-region is bias-only)
            bias_mms(0, bt0)
            w_mms(0, wst0, [hist[:, c, 0, :] for c in range(4)], last=True,
                  skip=("in",))

            next_proj = 1
            for t in range(steps):
                g = G[t % 2]
                rz_s = wk.tile([128, 256], F32, tag="rz")
                a_s = wk.tile([128, 128], F32, tag="a")
                b_s = wk.tile([128, 128], F32, tag="b")
                n16 = wk.tile([128, 128], F16, tag="n")
                t1 = wk.tile([128, 4, BD], F16, tag="t1")
                u16 = wk.tile([128, 4, BD], F16, tag="u")
                e16 = wk.tile([128, 4, BD], F16, tag="e")

                # bias matmuls for the NEXT step go first: no data deps, so
                # they fill the PE right after this step's e-matmuls drain
                # (emitted later they inherit a late coalesced wait).
                if t + 1 < steps:
                    bias_mms(t + 1, bt)

                # critical chain: rz -> a -> b -> n -> e.  The z gate's
                # weights/bias are negated host-side, so one fused sigmoid
                # over [r|z] yields [sig(r) | 1-z] with a single sem update
                # (separate Act ops get their updates coalesced, stalling a).
                nc.scalar.activation(rz_s, g[:, 0:256], SIG)
                z1mf = rz_s[:, 128:256]
                z1mf4 = z1mf.rearrange("p (q b) -> p q b", q=4)
                # a/b/n/e in q-halves: the first half reaches the PE while
                # the second is still in flight
                for h0, h1 in ((0, 64), (64, 128)):
                    nc.vector.tensor_mul(a_s[:, h0:h1], rz_s[:, h0:h1],
                                         g[:, 256 + h0:256 + h1])
                    nc.vector.tensor_add(b_s[:, h0:h1], a_s[:, h0:h1],
                                         g[:, 384 + h0:384 + h1])
                n4 = n16.rearrange("p (q b) -> p q b", q=4)
                nc.scalar.activation(n16[:, 0:64], b_s[:, 0:64], TANH)
                nc.scalar.activation(n16[:, 64:128], b_s[:, 64:128], TANH)
                # off-chain u path in halves on Pool (mixed f32*f16 keeps
                # DVE free for the a/b/e chain)
                nc.gpsimd.tensor_mul(t1[:, 0:2, :], z1mf4[:, 0:2, :],
                                     hist[:, 0:2, t, :])
                nc.gpsimd.tensor_sub(u16[:, 0:2, :], hist[:, 0:2, t, :],
                                     t1[:, 0:2, :])
                nc.gpsimd.tensor_mul(t1[:, 2:4, :], z1mf4[:, 2:4, :],
                                     hist[:, 2:4, t, :])
                nc.gpsimd.tensor_sub(u16[:, 2:4, :], hist[:, 2:4, t, :],
                                     t1[:, 2:4, :])
                nc.vector.tensor_mul(e16[:, 0:2, :], z1mf4[:, 0:2, :],
                                     n4[:, 0:2, :])
                nc.vector.tensor_mul(e16[:, 2:4, :], z1mf4[:, 2:4, :],
                                     n4[:, 2:4, :])
                # h join (off the recurrence cycle; feeds hist/proj/u-path)
                nc.vector.tensor_add(hist[:, :, t + 1, :], u16, e16)

                if t + 1 < steps:
                    u4 = [u16[:, c, :] for c in range(4)]
                    e4 = [e16[:, c, :] for c in range(4)]
                    w_mms(t + 1, wst, u4, last=False, cs=(0, 1))
                    w_mms(t + 1, wst, u4, last=False, cs=(2, 3))
                    w_mms(t + 1, wst, e4, last=False, cs=(0, 1))
                    w_mms(t + 1, wst, e4, last=True, cs=(2, 3))
                    if next_proj + PROJ_EVERY <= t:
                        proj_rows(next_proj, PROJ_EVERY)
                        next_proj += PROJ_EVERY

            while next_proj <= steps:
                nrows = min(PROJ_EVERY, steps + 1 - next_proj)
                proj_rows(next_proj, nrows)
                next_proj += nrows

        nc.sync.dma_start(
            out=out_d.rearrange("b v t -> v b t"),
            in_=stage.rearrange("p (b t) -> p b t", b=BD))
    nc.compile()
    return nc


_CACHE = {}


def _get_nc(steps: int):
    if steps not in _CACHE:
        _CACHE[steps] = _build(steps)
    return _CACHE[steps]


def _prep_inputs(feat, embed_table, w_ih, w_hh, b_ih, b_hh, proj_w, proj_b):
    f32 = np.float32
    f16 = np.float16
    w_ih = np.asarray(w_ih, f32)
    w_hh = np.asarray(w_hh, f32)
    b_ih = np.asarray(b_ih, f32)
    b_hh = np.asarray(b_hh, f32)
    # fused gate weights, gate-major order r, z, hn, in
    # z gate negated: sigmoid(z psum) then directly equals 1 - z
    Wc = np.concatenate([w_ih[:H] + w_hh[:H],
                         -(w_ih[H:2 * H] + w_hh[H:2 * H]),
                         w_hh[2 * H:],
                         w_ih[2 * H:]], 0)          # [4H, H]
    bc = np.concatenate([b_ih[:H] + b_hh[:H],
                         -(b_ih[H:2 * H] + b_hh[H:2 * H]),
                         b_hh[2 * H:],
                         b_ih[2 * H:]], 0)          # [4H]

    x0 = np.asarray(embed_table, f32)[0]
    gi0 = w_ih @ x0 + b_ih                          # [3H]
    bc0 = np.concatenate([gi0[:H] + b_hh[:H],
                          -(gi0[H:2 * H] + b_hh[H:2 * H]),
                          b_hh[2 * H:],
                          gi0[2 * H:]], 0)          # [4H]
    W0 = np.concatenate([w_hh[:H], -w_hh[H:2 * H]], 0)  # [2H, H] r,z step-0

    # stationary blocks: wst[kp, ((g*4+q)*4+c)*128 + m] = Wc[g*512+q*128+m,
    #                                                        c*128+kp]
    wst = np.empty((128, 64 * 128), f32)
    for g in range(4):
        for q in range(4):
            for c in range(4):
                blk = ((g * 4 + q) * 4 + c) * 128
                wst[:, blk:blk + 128] = Wc[g * 512 + q * 128:
                                           g * 512 + (q + 1) * 128,
                                           c * 128:(c + 1) * 128].T
    wst0 = np.empty((128, 32 * 128), f32)
    for g in range(2):
        for q in range(4):
            for c in range(4):
                blk = ((g * 4 + q) * 4 + c) * 128
                wst0[:, blk:blk + 128] = W0[g * 512 + q * 128:
                                            g * 512 + (q + 1) * 128,
                                            c * 128:(c + 1) * 128].T

    proj_w = np.asarray(proj_w, f32)                # [V, H]
    pjt = np.empty((128, 4 * VOCAB), f32)
    for c in range(4):
        pjt[:, c * VOCAB:(c + 1) * VOCAB] = proj_w[:, c * 128:(c + 1) * 128].T

    feat = np.asarray(feat, f32)
    common = {
        "wst": wst.astype(f16),
        "wst0": wst0.astype(f16),
        "bt": bc.reshape(1, 2048).astype(f16),
        "bt0": bc0.reshape(1, 2048).astype(f16),
        "ones": np.ones((1, BD), f16),
        "pjt": pjt.astype(f16),
        "pjb": np.asarray(proj_b, f32).reshape(VOCAB, 1),
    }
    maps = []
    for i in range(NCORES):
        fs = feat[i * BD:(i + 1) * BD]              # [BD, H]
        h0g = np.ascontiguousarray(
            fs.T.reshape(4, 128, BD).transpose(1, 0, 2).reshape(128, 128))
        maps.append(dict(common, h0=h0g.astype(f16)))
    return maps


def kernel(feat, embed_table, w_ih, w_hh, b_ih, b_hh, proj_w, proj_b,
           _trace=False):
    nc = _get_nc(STEPS)
    in_maps = _prep_inputs(feat, embed_table, w_ih, w_hh, b_ih, b_hh,
                           proj_w, proj_b)
    res = run_bass_kernel_spmd(nc, in_maps, list(range(NCORES)), trace=_trace)
    out = np.concatenate([res.results[i]["out"] for i in range(NCORES)], 0)
    if _trace:
        kernel.last_exec_time_ns = res.exec_time_ns
        kernel.last_results = res
    return out


# revision 22
# speedup vs baseline: 1.8802x; 1.0118x over previous
"""Trainium2 Bass kernel for nn_CaptionModel (GRU caption decoder).

Model: h0 = feat; x0 = embed[<SOS>]; 200 GRU steps where the output hidden
state is fed back as the next input (x_t = h_t for t >= 1), then a linear
projection of every hidden state to vocab logits, output [B, V, T].

Since x_t == h_t for t >= 1 the two GRU matmuls fuse into one 2048-wide
gate matmul G = h @ Wc.T + bc with Wc = [w_ih_r+w_hh_r; w_ih_z+w_hh_z;
w_hh_n; w_ih_n], gates r = sig(G0), z = sig(G1), n = tanh(G3 + r*G2),
h' = (1-z)*n + z*h.

Layout: GATE-MAJOR, fp16 matmuls. Each core holds batch BD=32. The PE
computes G^T [2048 gates -> 16 chunks of 128 partitions, 32 batch free]
with the 128x128 weight blocks as stationary operands and the (tiny)
hidden state as the moving operand: 16 bias rows + 128 weight matmuls of
32 moving rows each (~0.6us PE busy vs ~3.5us for weight-moving layouts;
stationary loads are pipelined). h' = u + e is never materialized on the
critical path: u = (1-z)*h (ready early) and e = (1-z)*n are fed to the
PE as TWO accumulating moving operands, removing the final join from the
recurrence cycle. z1m = 1-z comes directly from sigmoid(-zpre)
(scale=-1), and the u path (t1 = z1m*h, u = h - t1) runs off-chain on
Pool/DVE. The vocab projection is interleaved every 4 steps (lagged) to
fill PE idle time; hidden history is kept gate-major fp16 and projected
with the same stationary-weight trick.

Sharding: pure data parallelism, batch 256 -> 32 per core on 8 cores,
weights replicated.
"""

import os
from contextlib import ExitStack

import numpy as np

import concourse.bass as bass
import concourse.tile as tile
from concourse import bacc, mybir
from concourse.bass_utils import run_bass_kernel_spmd

B, H, VOCAB = 256, 512, 100
STEPS = int(os.environ.get("KERNEL_STEPS", "200"))
NCORES = 8
BD = B // NCORES  # 32
F16 = mybir.dt.float16
F32 = mybir.dt.float32
SIG = mybir.ActivationFunctionType.Sigmoid
TANH = mybir.ActivationFunctionType.Tanh

# gate order in both the weight blocks and the PSUM column regions
# r [0:128] z [128:256] hn [256:384] in [384:512]
GATES = ("r", "z", "hn", "in")
GI = {g: i for i, g in enumerate(GATES)}
PROJ_EVERY = 4


def _blk(g, q, c):
    return ((GI[g] * 4 + q) * 4 + c) * 128


def _build(steps: int):
    nc = bacc.Bacc("TRN2", target_bir_lowering=False, debug=False,
                   num_devices=NCORES)
    T1 = steps + 1

    wst_d = nc.dram_tensor("wst", [128, 64 * 128], F16, kind="ExternalInput").ap()
    wst0_d = nc.dram_tensor("wst0", [128, 32 * 128], F16, kind="ExternalInput").ap()
    h0_d = nc.dram_tensor("h0", [128, 128], F16, kind="ExternalInput").ap()
    bt_d = nc.dram_tensor("bt", [1, 2048], F16, kind="ExternalInput").ap()
    bt0_d = nc.dram_tensor("bt0", [1, 2048], F16, kind="ExternalInput").ap()
    ones_d = nc.dram_tensor("ones", [1, BD], F16, kind="ExternalInput").ap()
    pjt_d = nc.dram_tensor("pjt", [128, 4 * VOCAB], F16, kind="ExternalInput").ap()
    pjb_d = nc.dram_tensor("pjb", [VOCAB, 1], F32, kind="ExternalInput").ap()
    out_d = nc.dram_tensor("out", [BD, VOCAB, steps], F32,
                           kind="ExternalOutput").ap()

    with tile.TileContext(nc) as tc, ExitStack() as ctx:
        sg = ctx.enter_context(tc.tile_pool(name="sg", bufs=1))
        wk = ctx.enter_context(tc.tile_pool(name="wk", bufs=2))

        wst = sg.tile([128, 64 * 128], F16)
        nc.sync.dma_start(out=wst, in_=wst_d)
        wst0 = sg.tile([128, 32 * 128], F16)
        nc.sync.dma_start(out=wst0, in_=wst0_d)
        bt = sg.tile([1, 2048], F16)
        nc.sync.dma_start(out=bt, in_=bt_d)
        bt0 = sg.tile([1, 2048], F16)
        nc.sync.dma_start(out=bt0, in_=bt0_d)
        ones = sg.tile([1, BD], F16)
        nc.sync.dma_start(out=ones, in_=ones_d)
        pjt = sg.tile([128, 4 * VOCAB], F16)
        nc.sync.dma_start(out=pjt, in_=pjt_d)
        pjb = sg.tile([VOCAB, 1], F32)
        nc.sync.dma_start(out=pjb, in_=pjb_d)
        hist = sg.tile([128, 4, T1, BD], F16, name="hist")
        nc.sync.dma_start(out=hist[:, :, 0, :],
                          in_=h0_d.rearrange("p (q b) -> p q b", q=4))
        stage = sg.tile([VOCAB, BD * steps], F32, name="stage")

        with tc.tile_pool(name="gps", bufs=1, space="PSUM") as gpool, \
             tc.tile_pool(name="pps", bufs=2, space="PSUM") as ppool:
            G = [gpool.tile([128, 512], F32, tag=f"G{i}", name=f"G{i}")
                 for i in range(3)]

            # One PSUM accumulation group per G bank per step: start=True
            # zeroes the WHOLE 2KB zero region, so only the very first
            # matmul into the bank may carry start, and only the very last
            # carries stop.
            def bias_mms(t, bias):
                g = G[t % 3]
                first = True
                for gate in GATES:
                    for q in range(4):
                        nc.tensor.matmul(
                            g[:, GI[gate] * 128 + q * BD:
                              GI[gate] * 128 + (q + 1) * BD],
                            bias[:, GI[gate] * 512 + q * 128:
                                 GI[gate] * 512 + (q + 1) * 128],
                            ones, start=first, stop=False,
                            skip_group_check=True)
                        first = False

            def w_mms(t, w, rhs4, last, skip=(), cs=(0, 1, 2, 3)):
                g = G[t % 3]
                emitted = []
                for gate in GATES:
                    if gate in skip:
                        continue
                    for q in range(4):
                        for c in cs:
                            emitted.append((gate, q, c))
                for i, (gate, q, c) in enumerate(emitted):
                    col = GI[gate] * 128 + q * BD
                    if w is wst0 and gate in ("r", "z"):
                        wt, base = wst0, ((GI[gate] * 4 + q) * 4 + c) * 128
                    else:
                        wt, base = wst, _blk(gate, q, c)
                    nc.tensor.matmul(
                        g[:, col:col + BD],
                        wt[:, base:base + 128],
                        rhs4[c], start=False,
                        stop=(last and i == len(emitted) - 1),
                        skip_group_check=True)

            def proj_rows(r0, nrows):
                # P free dim iterates (b, t) so the stage (b-major,
                # t-minor) write and the final DMA stay contiguous in t
                Pfull = ppool.tile([VOCAB, 512], F32, tag="P", name="Pfull")
                P = Pfull[:, 0:BD * nrows]
                for c in range(4):
                    rhs = hist[:, c, r0:r0 + nrows, :].rearrange(
                        "p t b -> p b t")
                    nc.tensor.matmul(P, pjt[:, c * VOCAB:(c + 1) * VOCAB], rhs,
                                     start=(c == 0), stop=(c == 3))
                st_sl = stage.rearrange("p (b t) -> p b t", b=BD)[
                    :, :, r0 - 1:r0 - 1 + nrows]
                nc.vector.tensor_scalar_add(
                    st_sl, P.rearrange("p (b t) -> p b t", b=BD), pjb)

            # t=0: bias0 + w_hh matmuls on h0 (in-region is bias-only)
            bias_mms(0, bt0)
            w_mms(0, wst0, [hist[:, c, 0, :] for c in range(4)], last=True,
                  skip=("in",))

            # bias mms for step t+1 are emitted between step t's u and e
            # matmul batches: they execute inside the previous burst's
            # shadow instead of lengthening the critical burst
            if steps > 1:
                bias_mms(1, bt)
            next_proj = 1
            for t in range(steps):
                g = G[t % 3]
                rz_s = wk.tile([128, 256], F32, tag="rz")
                a_s = wk.tile([128, 128], F32, tag="a")
                b_s = wk.tile([128, 128], F32, tag="b")
                n16 = wk.tile([128, 128], F16, tag="n")
                t1 = wk.tile([128, 4, BD], F16, tag="t1")
                u16 = wk.tile([128, 4, BD], F16, tag="u")
                e16 = wk.tile([128, 4, BD], F16, tag="e")

                # critical cycle: z1m -> t1 -> u -> u-matmuls -> e-matmuls.
                # z1m = sigmoid(z psum) = 1-z (z weights negated host-side)
                # goes FIRST on Act: its update feeds Pool (u path) without
                # coalescing into r's Act->DVE update.
                z1mf = rz_s[:, 128:256]
                z1mf4 = z1mf.rearrange("p (q b) -> p q b", q=4)
                r_s = rz_s[:, 0:128]
                nc.scalar.activation(rz_s, g[:, 0:256], SIG)
                # u path: t1 = z1m*h kept in f32 so u = h - t1 rounds to
                # fp16 only once, relative to u's own magnitude.  First half
                # on Pool (fires right after z1m), second half on DVE after
                # the a/b chain.
                nc.gpsimd.tensor_mul(t1[:, 0:2, :], z1mf4[:, 0:2, :],
                                     hist[:, 0:2, t, :])
                nc.gpsimd.tensor_sub(u16[:, 0:2, :], hist[:, 0:2, t, :],
                                     t1[:, 0:2, :])
                # a/b/n/e in q-halves: the first half reaches the PE while
                # the second is still in flight
                for h0, h1 in ((0, 64), (64, 128)):
                    nc.vector.tensor_mul(a_s[:, h0:h1], r_s[:, h0:h1],
                                         g[:, 256 + h0:256 + h1])
                    nc.vector.tensor_add(b_s[:, h0:h1], a_s[:, h0:h1],
                                         g[:, 384 + h0:384 + h1])
                nc.vector.tensor_mul(t1[:, 2:4, :], z1mf4[:, 2:4, :],
                                     hist[:, 2:4, t, :])
                nc.vector.tensor_sub(u16[:, 2:4, :], hist[:, 2:4, t, :],
                                     t1[:, 2:4, :])
                n4 = n16.rearrange("p (q b) -> p q b", q=4)
                nc.scalar.activation(n16[:, 0:64], b_s[:, 0:64], TANH)
                nc.scalar.activation(n16[:, 64:128], b_s[:, 64:128], TANH)
                nc.vector.tensor_mul(e16[:, 0:2, :], z1mf4[:, 0:2, :],
                                     n4[:, 0:2, :])
                nc.vector.tensor_mul(e16[:, 2:4, :], z1mf4[:, 2:4, :],
                                     n4[:, 2:4, :])
                # h join (off the recurrence cycle; feeds hist/proj/u-path)
                nc.gpsimd.tensor_add(hist[:, :, t + 1, :], u16, e16)

                if t + 1 < steps:
                    u4 = [u16[:, c, :] for c in range(4)]
                    e4 = [e16[:, c, :] for c in range(4)]
                    w_mms(t + 1, wst, u4, last=False, cs=(0, 1))
                    w_mms(t + 1, wst, u4, last=False, cs=(2, 3))
                    if t + 2 < steps:
                        bias_mms(t + 2, bt)
                    w_mms(t + 1, wst, e4, last=False, cs=(0, 1))
                    w_mms(t + 1, wst, e4, last=True, cs=(2, 3))
                    if next_proj + PROJ_EVERY <= t:
                        proj_rows(next_proj, PROJ_EVERY)
                        next_proj += PROJ_EVERY

            while next_proj <= steps:
                nrows = min(PROJ_EVERY, steps + 1 - next_proj)
                proj_rows(next_proj, nrows)
                next_proj += nrows

        nc.sync.dma_start(
            out=out_d.rearrange("b v t -> v b t"),
            in_=stage.rearrange("p (b t) -> p b t", b=BD))
    nc.compile()
    return nc


_CACHE = {}


def _get_nc(steps: int):
    if steps not in _CACHE:
        _CACHE[steps] = _build(steps)
    return _CACHE[steps]


def _prep_inputs(feat, embed_table, w_ih, w_hh, b_ih, b_hh, proj_w, proj_b):
    f32 = np.float32
    f16 = np.float16
    w_ih = np.asarray(w_ih, f32)
    w_hh = np.asarray(w_hh, f32)
    b_ih = np.asarray(b_ih, f32)
    b_hh = np.asarray(b_hh, f32)
    # fused gate weights, gate-major order r, z, hn, in
    # z gate negated: sigmoid(z psum) then directly equals 1 - z
    Wc = np.concatenate([w_ih[:H] + w_hh[:H],
                         -(w_ih[H:2 * H] + w_hh[H:2 * H]),
                         w_hh[2 * H:],
                         w_ih[2 * H:]], 0)          # [4H, H]
    bc = np.concatenate([b_ih[:H] + b_hh[:H],
                         -(b_ih[H:2 * H] + b_hh[H:2 * H]),
                         b_hh[2 * H:],
                         b_ih[2 * H:]], 0)          # [4H]

    x0 = np.asarray(embed_table, f32)[0]
    gi0 = w_ih @ x0 + b_ih                          # [3H]
    bc0 = np.concatenate([gi0[:H] + b_hh[:H],
                          -(gi0[H:2 * H] + b_hh[H:2 * H]),
                          b_hh[2 * H:],
                          gi0[2 * H:]], 0)          # [4H]
    W0 = np.concatenate([w_hh[:H], -w_hh[H:2 * H]], 0)  # [2H, H] r,z step-0

    # stationary blocks: wst[kp, ((g*4+q)*4+c)*128 + m] = Wc[g*512+q*128+m,
    #                                                        c*128+kp]
    wst = np.empty((128, 64 * 128), f32)
    for g in range(4):
        for q in range(4):
            for c in range(4):
                blk = ((g * 4 + q) * 4 + c) * 128
                wst[:, blk:blk + 128] = Wc[g * 512 + q * 128:
                                           g * 512 + (q + 1) * 128,
                                           c * 128:(c + 1) * 128].T
    wst0 = np.empty((128, 32 * 128), f32)
    for g in range(2):
        for q in range(4):
            for c in range(4):
                blk = ((g * 4 + q) * 4 + c) * 128
                wst0[:, blk:blk + 128] = W0[g * 512 + q * 128:
                                            g * 512 + (q + 1) * 128,
                                            c * 128:(c + 1) * 128].T

    proj_w = np.asarray(proj_w, f32)                # [V, H]
    pjt = np.empty((128, 4 * VOCAB), f32)
    for c in range(4):
        pjt[:, c * VOCAB:(c + 1) * VOCAB] = proj_w[:, c * 128:(c + 1) * 128].T

    feat = np.asarray(feat, f32)
    common = {
        "wst": wst.astype(f16),
        "wst0": wst0.astype(f16),
        "bt": bc.reshape(1, 2048).astype(f16),
        "bt0": bc0.reshape(1, 2048).astype(f16),
        "ones": np.ones((1, BD), f16),
        "pjt": pjt.astype(f16),
        "pjb": np.asarray(proj_b, f32).reshape(VOCAB, 1),
    }
    maps = []
    for i in range(NCORES):
        fs = feat[i * BD:(i + 1) * BD]              # [BD, H]
        h0g = np.ascontiguousarray(
            fs.T.reshape(4, 128, BD).transpose(1, 0, 2).reshape(128, 128))
        maps.append(dict(common, h0=h0g.astype(f16)))
    return maps


def kernel(feat, embed_table, w_ih, w_hh, b_ih, b_hh, proj_w, proj_b,
           _trace=False):
    nc = _get_nc(STEPS)
    in_maps = _prep_inputs(feat, embed_table, w_ih, w_hh, b_ih, b_hh,
                           proj_w, proj_b)
    res = run_bass_kernel_spmd(nc, in_maps, list(range(NCORES)), trace=_trace)
    out = np.concatenate([res.results[i]["out"] for i in range(NCORES)], 0)
    if _trace:
        kernel.last_exec_time_ns = res.exec_time_ns
        kernel.last_results = res
    return out
